# revision 5
# baseline (speedup 1.0000x reference)
"""StyleGAN2-style modulated 3x3 conv (B=8, Ci=Co=512, H=W=32) on 8 TRN2 NeuronCores.

Sharding: data-parallel over batch, one sample per core (embarrassingly
parallel, no collectives).

Algorithm: Winograd F(2x2, 3x3). Per core:
  x ships pre-split by column parity (host): xt[j][par][r, q]
  mod:  parity planes P[par][j] = zero-padded(x * y_s)        (DVE TS)
  ct:   column-combine on parity planes (B^T d cols)          (DVE TT)
  v:    row-combine at stride-2 rows (B^T d B)                (DVE TT, 2x)
  M[xi,nu]   = sum_j U[xi,nu,j]^T @ V[j,xi,nu] (PE, fp32 PSUM, N=256)
  Y1[eta,nu] = xi-combine(M)                   (A^T M)        (DVE)
  Y[eta,mu]  = nu-combine(Y1)                  (A^T M A)      (DVE)
  ot[eta,mu,tile] = Y/rs + bias  (winograd layout; host descatters)
  rs = sqrt(sum_i ys2[i] w2[i,o] + eps)        demod (tiny PE matmuls)

Layout choices keep every hot DVE op on dense innermost runs so
tensor_tensor hits its 2x bf16 mode (the column-combine is inherently
one-element-offset and runs 1x, but on the smallest intermediate).
Output stays in winograd tile order so there is no strided scatter; the
host reshape is free.

Transformed weights U = G w G^T are computed host-side and streamed as
8 nu-paired 1MB slabs over three DMA queues in consumption order; demod
uses a separately shipped w2 = sum_k w_k^2.

Math note: the equal_lr scale s=(Ci*9)**-0.5 is folded out of both conv
and demod norm (eps compensated), so U/w2 come from the raw weights.
"""

import numpy as np
import ml_dtypes

import concourse.mybir as mybir
from concourse import bacc
from concourse.tile import TileContext
from concourse.bass_utils import run_bass_kernel_spmd

B = 8
CI = 512
CO = 512
H = W = 32
KK = 9
NCI = CI // 128
NCO = CO // 128
T = 16  # winograd tile grid (16x16 tiles of 2x2 outputs)
NPT = 256  # tiles per image = T*T
PR = 34  # padded rows
PW = 18  # parity-plane width (17 used + 1 pad for evenness)
EPS_EFF = 1e-8 * CI * KK

F32 = mybir.dt.float32
BF16 = mybir.dt.bfloat16
AF = mybir.ActivationFunctionType

# column-combine on parity planes. Plane 0 holds odd padded cols
# (p=2q+1, i.e. x even cols at idx 0..15, right-pad zero at idx 16);
# plane 1 holds even padded cols (p=2q: left-pad zero at idx 0, x odd
# cols at idx 1..16). Winograd col d_k = padded col 2tj+k:
#   d0 = plane1[tj], d1 = plane0[tj], d2 = plane1[tj+1], d3 = plane0[tj+1]
# nu0 = d0-d2, nu1 = d1+d2, nu2 = d2-d1, nu3 = d1-d3
# entries: (par_a, off_a, par_b, off_b, op)
CT_COMBINE = [
    (1, 0, 1, 1, "subtract"),
    (0, 0, 1, 1, "add"),
    (1, 1, 0, 0, "subtract"),
    (0, 0, 0, 1, "subtract"),
]
# row-combine (xi): rows a+2ti, b+2ti of ct
BT_ROWS = [(0, 2, "subtract"), (1, 2, "add"), (2, 1, "subtract"), (1, 3, "subtract")]


def build_nc():
    nc = bacc.Bacc("TRN2", target_bir_lowering=False, debug=False)

    # x: per j chunk [128, parity, 32, 16]
    x_ext = nc.declare_dram_parameter("x", [NCI, 128, 2, H, T], BF16, isOutput=False)
    yb_ext = nc.declare_dram_parameter("yb", [128, 2 * NCI], F32, isOutput=False)
    # transformed weights as nu-pairs: [jo, pair, ci_p, nu_in_pair, xi, j, co_c]
    u_ext = nc.declare_dram_parameter(
        "u", [NCO, 2, 128, 2, 4, NCI, 128], BF16, isOutput=False
    )
    # w2 = sum_k w_k^2: [ci_p, j, jo, co_c]
    w2_ext = nc.declare_dram_parameter("w2", [128, NCI, NCO, 128], BF16, isOutput=False)
    # out in winograd layout [jo, co_c, eta, mu, ti*16+tj]
    out_ext = nc.declare_dram_parameter("out", [NCO, 128, 2, 2, NPT], F32, isOutput=True)

    with TileContext(nc) as tc:
        with (
            tc.tile_pool(name="singles", bufs=1) as singles,
            tc.tile_pool(name="us", bufs=7) as us,
            tc.tile_pool(name="big", bufs=1) as big,
            tc.tile_pool(name="xin", bufs=1) as xin,
            tc.tile_pool(name="y1s", bufs=1) as y1s,
            tc.tile_pool(name="tmps", bufs=2) as tmps,
            tc.tile_pool(name="outs", bufs=2) as outs,
            tc.tile_pool(name="cps", bufs=3, space="PSUM") as cps,
            tc.tile_pool(name="dps", bufs=1, space="PSUM") as dps,
            tc.tile_pool(name="wps", bufs=1, space="PSUM") as wps,
        ):
            xt_sb = [
                xin.tile([128, 2, H, T], BF16, tag=f"x{j}", name=f"xt{j}")
                for j in range(NCI)
            ]
            yb_sb = singles.tile([128, 2 * NCI], F32)
            w2_sb = singles.tile([128, NCI, NCO, 128], BF16)

            # parity planes + transform intermediates
            # pl[par][j]: [34 rows, 18]; ct[j, nu]: [34 rows, 16];
            # v[j, xi, nu]: [256]
            pl = big.tile([128, 2, NCI, PR, PW], BF16)
            ct_sb = big.tile([128, NCI, 4, PR, T], BF16)
            v_sb = big.tile([128, NCI, 4, 4, NPT], BF16)

            # ---- padding memsets first so the gpsimd queue clears them
            # before anything else (mod depends on them) ----
            nc.gpsimd.memset(pl[:, :, :, 0, :], 0.0)
            nc.gpsimd.memset(pl[:, :, :, PR - 1, :], 0.0)
            nc.gpsimd.memset(pl[:, 0, :, 1 : PR - 1, T], 0.0)
            nc.gpsimd.memset(pl[:, 1, :, 1 : PR - 1, 0], 0.0)

            # ---- input DMAs, priority order per queue ----
            u_sb = {}  # (jo, pair) -> [128, 2nu, 4xi, 4j, 128co]

            def udma(engine, jo, p):
                t = us.tile([128, 2, 4, NCI, 128], BF16, tag="u", name=f"u{jo}{p}")
                engine.dma_start(out=t, in_=u_ext[jo, p])
                u_sb[(jo, p)] = t

            # consumption order of u pairs: (0,0) (1,0) (2,0) (3,0) (0,1)...
            # sync:   x0 x1 w2 (1,0) (0,1) (3,1)
            # scalar: yb x2 x3 (2,0) (1,1)         then out halves
            # gpsimd: memsets (0,0) (3,0) (2,1)    then out halves
            nc.scalar.dma_start(out=yb_sb, in_=yb_ext[:, :])
            nc.sync.dma_start(out=xt_sb[0], in_=x_ext[0])
            udma(nc.gpsimd, 0, 0)
            nc.sync.dma_start(out=xt_sb[1], in_=x_ext[1])
            nc.scalar.dma_start(out=xt_sb[2], in_=x_ext[2])
            nc.scalar.dma_start(out=xt_sb[3], in_=x_ext[3])
            nc.sync.dma_start(out=w2_sb, in_=w2_ext[:, :])
            udma(nc.gpsimd, 3, 0)
            udma(nc.sync, 1, 0)
            udma(nc.scalar, 2, 0)
            udma(nc.sync, 0, 1)
            udma(nc.gpsimd, 2, 1)
            udma(nc.scalar, 1, 1)
            udma(nc.sync, 3, 1)

            # ---- PE warm-up (HAM clock gate) ----
            warm_lhs = singles.tile([128, 1], BF16)
            nc.vector.memset(warm_lhs, 1.0)
            warm_rhs = singles.tile([128, 512], BF16)
            nc.vector.memset(warm_rhs, 0.5)
            warm_ps = wps.tile([1, 512], F32)
            N_WARM = 16
            for i in range(N_WARM):
                nc.tensor.matmul(
                    out=warm_ps,
                    lhsT=warm_lhs,
                    rhs=warm_rhs,
                    start=(i == 0),
                    stop=(i == N_WARM - 1),
                )

            eps_sb = singles.tile([128, 1], F32)
            nc.vector.memset(eps_sb, EPS_EFF)

            def mod(j):
                # plane0 (data at idx 0..15) from x even cols; plane1 at 1..16
                nc.vector.tensor_scalar(
                    out=pl[:, 0, j, 1 : H + 1, 0:T],
                    in0=xt_sb[j][:, 0],
                    scalar1=yb_sb[:, j : j + 1],
                    scalar2=None,
                    op0=mybir.AluOpType.mult,
                )
                nc.vector.tensor_scalar(
                    out=pl[:, 1, j, 1 : H + 1, 1 : T + 1],
                    in0=xt_sb[j][:, 1],
                    scalar1=yb_sb[:, j : j + 1],
                    scalar2=None,
                    op0=mybir.AluOpType.mult,
                )

            def ctop(nu, j=None):
                pa, oa, pb, ob, op = CT_COMBINE[nu]
                jj = slice(None) if j is None else slice(j, j + 1)
                nc.vector.tensor_tensor(
                    out=ct_sb[:, jj, nu],
                    in0=pl[:, pa, jj, :, oa : oa + T],
                    in1=pl[:, pb, jj, :, ob : ob + T],
                    op=getattr(mybir.AluOpType, op),
                )

            def vop(xi, nu):
                a, b, op = BT_ROWS[xi]
                nc.vector.tensor_tensor(
                    out=v_sb[:, :, xi, nu],
                    in0=ct_sb[:, :, nu, a : min(a + 2 * T, PR) : 2, :],
                    in1=ct_sb[:, :, nu, b : min(b + 2 * T, PR) : 2, :],
                    op=getattr(mybir.AluOpType, op),
                )

            for j in range(NCI):
                mod(j)
                ctop(0, j)
            ys2_sb = singles.tile([128, NCI], BF16)
            nc.vector.tensor_mul(ys2_sb, yb_sb[:, 0:NCI], yb_sb[:, 0:NCI])
            for xi in range(4):
                vop(xi, 0)
            for nu in range(1, 4):
                ctop(nu)
                for xi in range(4):
                    vop(xi, nu)

            # ---- demod: PE matmuls + sqrt + reciprocal, all early ----
            rs_sb = singles.tile([128, NCO], F32)
            xs2_ps = dps.tile([128, NCO], F32)
            for jo in range(NCO):
                for j in range(NCI):
                    nc.tensor.matmul(
                        out=xs2_ps[:, jo : jo + 1],
                        lhsT=w2_sb[:, j, jo],
                        rhs=ys2_sb[:, j : j + 1],
                        start=(j == 0),
                        stop=(j == NCI - 1),
                    )
            nc.scalar.activation(out=rs_sb, in_=xs2_ps, func=AF.Sqrt, bias=eps_sb)
            nc.vector.reciprocal(out=rs_sb, in_=rs_sb)

            # per-jo evacuated M (bf16): [128, 4nu, 4xi, 256]
            m_sb = [
                y1s.tile([128, 4, 4, NPT], BF16, tag=f"m_{jo}", name=f"m_{jo}")
                for jo in range(NCO)
            ]
            # per-jo Y1 (bf16): [128, 2eta, 4nu, 256]
            y1_sb = [
                y1s.tile([128, 2, 4, NPT], BF16, tag=f"y1_{jo}", name=f"y1_{jo}")
                for jo in range(NCO)
            ]
            # per-jo Y (bf16): [128, 2eta, 2mu, 256]
            yt_sb = [
                y1s.tile([128, 2, 2, NPT], BF16, tag=f"yt_{jo}", name=f"yt_{jo}")
                for jo in range(NCO)
            ]
            ot_sb = [
                outs.tile([128, 2, 2, NPT], F32, tag=f"ot{jo % 2}", name=f"ot{jo}")
                for jo in range(NCO)
            ]

            def unit(jo, nu):
                # M[xi] for this (jo, nu): 16 matmuls N=256, xi-outer so each
                # xi's accumulation chain is contiguous; one ACT op evacuates
                # all 4 xi slices to SBUF bf16.
                ps = cps.tile([128, 4, NPT], F32, tag="ups")
                u = u_sb[(jo, nu // 2)]
                for xi in range(4):
                    for j in range(NCI):
                        nc.tensor.matmul(
                            out=ps[:, xi],
                            lhsT=u[:, nu % 2, xi, j],
                            rhs=v_sb[:, j, xi, nu],
                            start=(j == 0),
                            stop=(j == NCI - 1),
                        )
                nc.scalar.activation(out=m_sb[jo][:, nu], in_=ps, func=AF.Copy)

            TT = nc.vector.tensor_tensor
            ADD = mybir.AluOpType.add
            SUB = mybir.AluOpType.subtract

            def stage1(jo, n0, n1):
                # Y1[0,nu] = M0+M1+M2 ; Y1[1,nu] = M1-M2-M3 over nu in [n0,n1)
                m = m_sb[jo]
                y1 = y1_sb[jo]
                t = tmps.tile([128, 4, NPT], BF16, tag="t1", name="t")
                TT(out=t[:, n0:n1], in0=m[:, n0:n1, 0], in1=m[:, n0:n1, 1], op=ADD)
                TT(out=y1[:, 0, n0:n1], in0=t[:, n0:n1], in1=m[:, n0:n1, 2], op=ADD)
                TT(out=t[:, n0:n1], in0=m[:, n0:n1, 1], in1=m[:, n0:n1, 2], op=SUB)
                TT(out=y1[:, 1, n0:n1], in0=t[:, n0:n1], in1=m[:, n0:n1, 3], op=SUB)

            def stage2(jo, mu):
                # Y[:,mu0] = Y1n0+Y1n1+Y1n2 ; Y[:,mu1] = Y1n1-Y1n2-Y1n3
                y1 = y1_sb[jo]
                yt = yt_sb[jo]
                op = ADD if mu == 0 else SUB
                na, nb, ncol = (0, 1, 2) if mu == 0 else (1, 2, 3)
                t = tmps.tile([128, 2, NPT], BF16, tag="t2", name="t")
                TT(out=t, in0=y1[:, :, na], in1=y1[:, :, nb], op=op)
                TT(out=yt[:, :, mu], in0=t, in1=y1[:, :, ncol], op=op)

            def finish(jo):
                # ot = yt*rs + bias in winograd layout (no strided scatter);
                # ACT takes eta=0 and DVE eta=1 to split the load
                yt = yt_sb[jo]
                ot = ot_sb[jo]
                nc.scalar.activation(
                    out=ot[:, 0],
                    in_=yt[:, 0],
                    func=AF.Identity,
                    bias=yb_sb[:, NCI + jo : NCI + jo + 1],
                    scale=rs_sb[:, jo : jo + 1],
                )
                nc.vector.tensor_scalar(
                    out=ot[:, 1],
                    in0=yt[:, 1],
                    scalar1=rs_sb[:, jo : jo + 1],
                    scalar2=yb_sb[:, NCI + jo : NCI + jo + 1],
                    op0=mybir.AluOpType.mult,
                    op1=mybir.AluOpType.add,
                )

            # ---- PE stream: nu-outer rounds; per-jo combine work staggered:
            # nu0-2 parts after round 2, nu3 parts + output after each
            # round-3 unit ----
            for jo in range(NCO):
                unit(jo, 0)
            for jo in range(NCO):
                unit(jo, 1)
            for jo in range(NCO):
                unit(jo, 2)
                stage1(jo, 0, 3)
            # out DMA queue pairs (eta0, eta1) per jo
            out_q = [
                (nc.scalar, nc.gpsimd),
                (nc.sync, nc.scalar),
                (nc.gpsimd, nc.sync),
                (nc.scalar, nc.gpsimd),
            ]
            for jo in range(NCO):
                unit(jo, 3)
                stage1(jo, 3, 4)
                stage2(jo, 0)
                stage2(jo, 1)
                finish(jo)
                q0, q1 = out_q[jo]
                q0.dma_start(out=out_ext[jo][:, 0], in_=ot_sb[jo][:, 0])
                q1.dma_start(out=out_ext[jo][:, 1], in_=ot_sb[jo][:, 1])

            warm_sink = singles.tile([1, 1], F32)
            nc.vector.tensor_copy(out=warm_sink, in_=warm_ps[0:1, 0:1])
    nc.compile()
    return nc


_NC_CACHE = None


def _get_nc():
    global _NC_CACHE
    if _NC_CACHE is None:
        _NC_CACHE = build_nc()
    return _NC_CACHE


_G = np.array(
    [[1, 0, 0], [0.5, 0.5, 0.5], [0.5, -0.5, 0.5], [0, 0, 1]], np.float64
)


def _prep_inputs(x, y_s, weight, bias):
    w = weight.astype(np.float64)
    # U[xi, nu, co, ci] = G w G^T (input-independent weight transform)
    u = np.einsum("xa,nb,oiab->xnoi", _G, _G, w)
    # arrange to [jo, pair, ci_p, nu_in_pair, xi, j, co_c]
    u7 = u.reshape(4, 2, 2, NCO, 128, NCI, 128).transpose(3, 1, 6, 2, 0, 5, 4)
    u_arr = np.ascontiguousarray(u7).astype(ml_dtypes.bfloat16)
    w2 = (w**2).sum(axis=(2, 3))  # [co, ci]
    # [ci_p, j, jo, co_c]
    w2_arr = np.ascontiguousarray(
        w2.reshape(NCO, 128, NCI, 128).transpose(3, 2, 0, 1)
    ).astype(ml_dtypes.bfloat16)
    in_maps = []
    # x parity split: [j, ci_p, parity, r, q]; parity0 = even cols,
    # parity1 = odd cols
    xb = x.reshape(B, NCI, 128, H, T, 2)
    for b in range(B):
        yb = np.empty((128, 2 * NCI), np.float32)
        yb[:, :NCI] = y_s[b].reshape(NCI, 128).T
        yb[:, NCI:] = bias.reshape(NCO, 128).T
        xs = np.ascontiguousarray(xb[b].transpose(0, 1, 4, 2, 3)).astype(
            ml_dtypes.bfloat16
        )
        in_maps.append({"x": xs, "yb": yb, "u": u_arr, "w2": w2_arr})
    return in_maps


def _install_trace_support():
    """Dev-only: register the axon NTFF profiling hook + disable the
    remote artifact upload so trace=True works in this container."""
    import sys
    import types

    import concourse.bass_utils as bu

    bu.upload_artifacts = lambda tmpdir: "local://" + str(tmpdir)
    if "antenv.axon_hooks" in sys.modules:
        return
    try:
        from trn_agent_boot.trn_boot import _ntff_profile_via_ctypes

        hook = _ntff_profile_via_ctypes("/opt/axon/libaxon_pjrt.so")
    except Exception:
        return
    mod = types.ModuleType("antenv.axon_hooks")
    mod.get_axon_ntff_profile_hook = lambda: hook
    mod.set_axon_ntff_profile_hook = lambda h: None
    sys.modules["antenv.axon_hooks"] = mod


def run(x, y_s, weight, bias, trace=False, tmpdir=None):
    nc = _get_nc()
    if trace:
        _install_trace_support()
    in_maps = _prep_inputs(x, y_s, weight, bias)
    res = run_bass_kernel_spmd(
        nc, in_maps, core_ids=list(range(B)), trace=trace, tmpdir=tmpdir
    )
    # descatter winograd layout: [jo, co_c, eta, mu, ti, tj] -> [co, h, w]
    out = np.stack(
        [
            res.results[b]["out"]
            .reshape(NCO, 128, 2, 2, T, T)
            .transpose(0, 1, 4, 2, 5, 3)
            .reshape(CO, H, W)
            for b in range(B)
        ]
    ).astype(np.float32)
    return out, res


def kernel(x, y_s, weight, bias):
    out, _ = run(
        np.asarray(x, dtype=np.float32),
        np.asarray(y_s, dtype=np.float32),
        np.asarray(weight, dtype=np.float32),
        np.asarray(bias, dtype=np.float32),
    )
    return out


# revision 8
# speedup vs baseline: 1.1213x; 1.1213x over previous
"""StyleGAN2-style modulated 3x3 conv (B=8, Ci=Co=512, H=W=32) on 8 TRN2 NeuronCores.

Sharding: data-parallel over batch, one sample per core (embarrassingly
parallel, no collectives).

Algorithm: Winograd F(2x2, 3x3). Per core:
  x ships pre-split by column parity (host): xt[j][par][r, q]
  mod:  parity planes P[par][j] = zero-padded(x * y_s)        (DVE TS)
  ct:   column-combine on parity planes (B^T d cols)          (DVE TT)
  v:    row-combine at stride-2 rows (B^T d B)                (DVE TT, 2x)
  M[xi,nu]   = sum_j U[xi,nu,j]^T @ V[j,xi,nu] (PE, fp32 PSUM, N=256)
  Y1[eta,nu] = xi-combine(M)                   (A^T M)        (DVE)
  Y[eta,mu]  = nu-combine(Y1)                  (A^T M A)      (DVE)
  ot[eta,mu,tile] = Y/rs + bias  (winograd layout; host descatters)
  rs = sqrt(sum_i ys2[i] w2[i,o] + eps)        demod (tiny PE matmuls)

Layout choices keep every hot DVE op on dense innermost runs so
tensor_tensor hits its 2x bf16 mode (the column-combine is inherently
one-element-offset and runs 1x, but on the smallest intermediate).
Output stays in winograd tile order so there is no strided scatter; the
host reshape is free.

Transformed weights U = G w G^T are computed host-side and streamed as
8 nu-paired 1MB slabs over three DMA queues in consumption order; demod
uses a separately shipped w2 = sum_k w_k^2.

Math note: the equal_lr scale s=(Ci*9)**-0.5 is folded out of both conv
and demod norm (eps compensated), so U/w2 come from the raw weights.
"""

import numpy as np
import ml_dtypes

import concourse.mybir as mybir
from concourse import bacc
from concourse.tile import TileContext
from concourse.bass_utils import run_bass_kernel_spmd

B = 8
CI = 512
CO = 512
H = W = 32
KK = 9
NCI = CI // 128
NCO = CO // 128
T = 16  # winograd tile grid (16x16 tiles of 2x2 outputs)
NPT = 256  # tiles per image = T*T
PR = 34  # padded rows
PW = 18  # parity-plane width (17 used + 1 pad for evenness)
EPS_EFF = 1e-8 * CI * KK

F32 = mybir.dt.float32
BF16 = mybir.dt.bfloat16
AF = mybir.ActivationFunctionType

# column-combine on parity planes. Plane 0 holds odd padded cols
# (p=2q+1, i.e. x even cols at idx 0..15, right-pad zero at idx 16);
# plane 1 holds even padded cols (p=2q: left-pad zero at idx 0, x odd
# cols at idx 1..16). Winograd col d_k = padded col 2tj+k:
#   d0 = plane1[tj], d1 = plane0[tj], d2 = plane1[tj+1], d3 = plane0[tj+1]
# nu0 = d0-d2, nu1 = d1+d2, nu2 = d2-d1, nu3 = d1-d3
# entries: (par_a, off_a, par_b, off_b, op)
CT_COMBINE = [
    (1, 0, 1, 1, "subtract"),
    (0, 0, 1, 1, "add"),
    (1, 1, 0, 0, "subtract"),
    (0, 0, 0, 1, "subtract"),
]
# row-combine (xi): rows a+2ti, b+2ti of ct
BT_ROWS = [(0, 2, "subtract"), (1, 2, "add"), (2, 1, "subtract"), (1, 3, "subtract")]


def build_nc():
    nc = bacc.Bacc("TRN2", target_bir_lowering=False, debug=False)

    # x: per j chunk [128, parity, 32, 16]
    x_ext = nc.declare_dram_parameter("x", [NCI, 128, 2, H, T], BF16, isOutput=False)
    yb_ext = nc.declare_dram_parameter("yb", [128, 2 * NCI], F32, isOutput=False)
    # transformed weights: [jo, nu, ci_p, xi, j, co_c]
    u_ext = nc.declare_dram_parameter(
        "u", [NCO, 4, 128, 4, NCI, 128], BF16, isOutput=False
    )
    # w2 = sum_k w_k^2: [ci_p, j, jo, co_c]
    w2_ext = nc.declare_dram_parameter("w2", [128, NCI, NCO, 128], BF16, isOutput=False)
    # out in winograd layout [jo, co_c, eta, mu, ti*16+tj], bf16 (host widens)
    out_ext = nc.declare_dram_parameter(
        "out", [NCO, 128, 2, 2, NPT], BF16, isOutput=True
    )

    with TileContext(nc) as tc:
        with (
            tc.tile_pool(name="singles", bufs=1) as singles,
            tc.tile_pool(name="us", bufs=10) as us,
            tc.tile_pool(name="big", bufs=1) as big,
            tc.tile_pool(name="xin", bufs=1) as xin,
            tc.tile_pool(name="y1s", bufs=1) as y1s,
            tc.tile_pool(name="tmps", bufs=2) as tmps,
            tc.tile_pool(name="outs", bufs=2) as outs,
            tc.tile_pool(name="cps", bufs=3, space="PSUM") as cps,
            tc.tile_pool(name="dps", bufs=1, space="PSUM") as dps,
            tc.tile_pool(name="wps", bufs=1, space="PSUM") as wps,
        ):
            xt_sb = [
                xin.tile([128, 2, H, T], BF16, tag=f"x{j}", name=f"xt{j}")
                for j in range(NCI)
            ]
            yb_sb = singles.tile([128, 2 * NCI], F32)
            w2_sb = singles.tile([128, NCI, NCO, 128], BF16)

            # parity planes + transform intermediates
            # pl[par][j]: [34 rows, 18]; ct[j, nu]: [34 rows, 16];
            # v[j, xi, nu]: [256]
            pl = big.tile([128, 2, NCI, PR, PW], BF16)
            ct_sb = big.tile([128, NCI, 4, PR, T], BF16)
            v_sb = big.tile([128, NCI, 4, 4, NPT], BF16)

            # ---- padding memsets first so the gpsimd queue clears them
            # before anything else (mod depends on them) ----
            nc.gpsimd.memset(pl[:, :, :, 0, :], 0.0)
            nc.gpsimd.memset(pl[:, :, :, PR - 1, :], 0.0)
            nc.gpsimd.memset(pl[:, 0, :, 1 : PR - 1, T], 0.0)
            nc.gpsimd.memset(pl[:, 1, :, 1 : PR - 1, 0], 0.0)

            # ---- input DMAs, priority order per queue ----
            u_sb = {}  # (jo, nu) -> [128, 4xi, 4j, 128co]

            def udma(engine, jo, nu):
                t = us.tile([128, 4, NCI, 128], BF16, tag="u", name=f"u{jo}{nu}")
                engine.dma_start(out=t, in_=u_ext[jo, nu])
                u_sb[(jo, nu)] = t

            # x first (gates the input transform), then u slabs interleaved
            # across all three queues in consumption order k = 4*nu + jo:
            #   scalar: yb x2 x3 k0 k3 k6 k9 k12 k15    then out jo0, jo2
            #   sync:   x0 x1    k2 k5 k8 k11 k14       then out jo1, jo3b
            #   gpsimd: w2       k1 k4 k7 k10 k13       then out jo3a
            nc.scalar.dma_start(out=yb_sb, in_=yb_ext[:, :])
            nc.sync.dma_start(out=xt_sb[0], in_=x_ext[0])
            nc.scalar.dma_start(out=xt_sb[2], in_=x_ext[2])
            nc.gpsimd.dma_start(out=w2_sb, in_=w2_ext[:, :])
            nc.sync.dma_start(out=xt_sb[1], in_=x_ext[1])
            nc.scalar.dma_start(out=xt_sb[3], in_=x_ext[3])
            q_of = {0: nc.scalar, 1: nc.gpsimd, 2: nc.sync}
            for k in range(16):
                udma(q_of[k % 3], k % 4, k // 4)

            # ---- PE warm-up (HAM clock gate) ----
            warm_lhs = singles.tile([128, 1], BF16)
            nc.vector.memset(warm_lhs, 1.0)
            warm_rhs = singles.tile([128, 512], BF16)
            nc.vector.memset(warm_rhs, 0.5)
            warm_ps = wps.tile([1, 512], F32)
            N_WARM = 16
            for i in range(N_WARM):
                nc.tensor.matmul(
                    out=warm_ps,
                    lhsT=warm_lhs,
                    rhs=warm_rhs,
                    start=(i == 0),
                    stop=(i == N_WARM - 1),
                )

            eps_sb = singles.tile([128, 1], F32)
            nc.vector.memset(eps_sb, EPS_EFF)

            def mod(j):
                # plane0 (data at idx 0..15) from x even cols; plane1 at 1..16
                nc.vector.tensor_scalar(
                    out=pl[:, 0, j, 1 : H + 1, 0:T],
                    in0=xt_sb[j][:, 0],
                    scalar1=yb_sb[:, j : j + 1],
                    scalar2=None,
                    op0=mybir.AluOpType.mult,
                )
                nc.vector.tensor_scalar(
                    out=pl[:, 1, j, 1 : H + 1, 1 : T + 1],
                    in0=xt_sb[j][:, 1],
                    scalar1=yb_sb[:, j : j + 1],
                    scalar2=None,
                    op0=mybir.AluOpType.mult,
                )

            def ctop(nu, j=None):
                pa, oa, pb, ob, op = CT_COMBINE[nu]
                jj = slice(None) if j is None else slice(j, j + 1)
                nc.vector.tensor_tensor(
                    out=ct_sb[:, jj, nu],
                    in0=pl[:, pa, jj, :, oa : oa + T],
                    in1=pl[:, pb, jj, :, ob : ob + T],
                    op=getattr(mybir.AluOpType, op),
                )

            def vop(xi, nu):
                a, b, op = BT_ROWS[xi]
                nc.vector.tensor_tensor(
                    out=v_sb[:, :, xi, nu],
                    in0=ct_sb[:, :, nu, a : min(a + 2 * T, PR) : 2, :],
                    in1=ct_sb[:, :, nu, b : min(b + 2 * T, PR) : 2, :],
                    op=getattr(mybir.AluOpType, op),
                )

            for j in range(NCI):
                mod(j)
                ctop(0, j)
            ys2_sb = singles.tile([128, NCI], BF16)
            nc.vector.tensor_mul(ys2_sb, yb_sb[:, 0:NCI], yb_sb[:, 0:NCI])
            for xi in range(4):
                vop(xi, 0)
            for nu in range(1, 4):
                ctop(nu)
                for xi in range(4):
                    vop(xi, nu)

            # ---- demod: PE matmuls + sqrt + reciprocal, all early ----
            rs_sb = singles.tile([128, NCO], F32)
            xs2_ps = dps.tile([128, NCO], F32)
            for jo in range(NCO):
                for j in range(NCI):
                    nc.tensor.matmul(
                        out=xs2_ps[:, jo : jo + 1],
                        lhsT=w2_sb[:, j, jo],
                        rhs=ys2_sb[:, j : j + 1],
                        start=(j == 0),
                        stop=(j == NCI - 1),
                    )
            nc.scalar.activation(out=rs_sb, in_=xs2_ps, func=AF.Sqrt, bias=eps_sb)
            nc.vector.reciprocal(out=rs_sb, in_=rs_sb)

            # per-jo evacuated M (bf16): [128, 4nu, 4xi, 256]
            m_sb = [
                y1s.tile([128, 4, 4, NPT], BF16, tag=f"m_{jo}", name=f"m_{jo}")
                for jo in range(NCO)
            ]
            # per-jo Y1 (bf16): [128, 2eta, 4nu, 256]
            y1_sb = [
                y1s.tile([128, 2, 4, NPT], BF16, tag=f"y1_{jo}", name=f"y1_{jo}")
                for jo in range(NCO)
            ]
            # per-jo Y (bf16): [128, 2eta, 2mu, 256]
            yt_sb = [
                y1s.tile([128, 2, 2, NPT], BF16, tag=f"yt_{jo}", name=f"yt_{jo}")
                for jo in range(NCO)
            ]
            ot_sb = [
                outs.tile([128, 2, 2, NPT], BF16, tag=f"ot{jo % 2}", name=f"ot{jo}")
                for jo in range(NCO)
            ]

            def unit(jo, nu):
                # M[xi] for this (jo, nu): 16 matmuls N=256, xi-outer so each
                # xi's accumulation chain is contiguous; one ACT op evacuates
                # all 4 xi slices to SBUF bf16.
                ps = cps.tile([128, 4, NPT], F32, tag="ups")
                u = u_sb[(jo, nu)]
                for xi in range(4):
                    for j in range(NCI):
                        nc.tensor.matmul(
                            out=ps[:, xi],
                            lhsT=u[:, xi, j],
                            rhs=v_sb[:, j, xi, nu],
                            start=(j == 0),
                            stop=(j == NCI - 1),
                        )
                nc.scalar.activation(out=m_sb[jo][:, nu], in_=ps, func=AF.Copy)

            TT = nc.vector.tensor_tensor
            ADD = mybir.AluOpType.add
            SUB = mybir.AluOpType.subtract

            def stage1(jo, n0, n1):
                # Y1[0,nu] = M0+M1+M2 ; Y1[1,nu] = M1-M2-M3 over nu in [n0,n1)
                m = m_sb[jo]
                y1 = y1_sb[jo]
                t = tmps.tile([128, 4, NPT], BF16, tag="t1", name="t")
                TT(out=t[:, n0:n1], in0=m[:, n0:n1, 0], in1=m[:, n0:n1, 1], op=ADD)
                TT(out=y1[:, 0, n0:n1], in0=t[:, n0:n1], in1=m[:, n0:n1, 2], op=ADD)
                TT(out=t[:, n0:n1], in0=m[:, n0:n1, 1], in1=m[:, n0:n1, 2], op=SUB)
                TT(out=y1[:, 1, n0:n1], in0=t[:, n0:n1], in1=m[:, n0:n1, 3], op=SUB)

            def stage2(jo, mu):
                # Y[:,mu0] = Y1n0+Y1n1+Y1n2 ; Y[:,mu1] = Y1n1-Y1n2-Y1n3
                y1 = y1_sb[jo]
                yt = yt_sb[jo]
                op = ADD if mu == 0 else SUB
                na, nb, ncol = (0, 1, 2) if mu == 0 else (1, 2, 3)
                t = tmps.tile([128, 2, NPT], BF16, tag="t2", name="t")
                TT(out=t, in0=y1[:, :, na], in1=y1[:, :, nb], op=op)
                TT(out=yt[:, :, mu], in0=t, in1=y1[:, :, ncol], op=op)

            def finish(jo):
                # ot = yt*rs + bias in winograd layout (no strided scatter);
                # ACT takes eta=0 and DVE eta=1 to split the load
                yt = yt_sb[jo]
                ot = ot_sb[jo]
                nc.scalar.activation(
                    out=ot[:, 0],
                    in_=yt[:, 0],
                    func=AF.Identity,
                    bias=yb_sb[:, NCI + jo : NCI + jo + 1],
                    scale=rs_sb[:, jo : jo + 1],
                )
                nc.vector.tensor_scalar(
                    out=ot[:, 1],
                    in0=yt[:, 1],
                    scalar1=rs_sb[:, jo : jo + 1],
                    scalar2=yb_sb[:, NCI + jo : NCI + jo + 1],
                    op0=mybir.AluOpType.mult,
                    op1=mybir.AluOpType.add,
                )

            # ---- PE stream: nu-outer rounds; per-jo combine work staggered:
            # nu0-2 parts after round 2, nu3 parts + output after each
            # round-3 unit ----
            for jo in range(NCO):
                unit(jo, 0)
            for jo in range(NCO):
                unit(jo, 1)
            for jo in range(NCO):
                unit(jo, 2)
                stage1(jo, 0, 3)
            out_whole_q = [nc.scalar, nc.sync, nc.scalar]
            for jo in range(NCO):
                unit(jo, 3)
                stage1(jo, 3, 4)
                stage2(jo, 0)
                stage2(jo, 1)
                finish(jo)
                if jo < 3:
                    out_whole_q[jo].dma_start(out=out_ext[jo], in_=ot_sb[jo])
                else:
                    nc.gpsimd.dma_start(out=out_ext[jo][:, 0], in_=ot_sb[jo][:, 0])
                    nc.sync.dma_start(out=out_ext[jo][:, 1], in_=ot_sb[jo][:, 1])

            warm_sink = singles.tile([1, 1], F32)
            nc.vector.tensor_copy(out=warm_sink, in_=warm_ps[0:1, 0:1])
    nc.compile()
    return nc


_NC_CACHE = None


def _get_nc():
    global _NC_CACHE
    if _NC_CACHE is None:
        _NC_CACHE = build_nc()
    return _NC_CACHE


_G = np.array(
    [[1, 0, 0], [0.5, 0.5, 0.5], [0.5, -0.5, 0.5], [0, 0, 1]], np.float64
)


def _prep_inputs(x, y_s, weight, bias):
    w = weight.astype(np.float64)
    # U[xi, nu, co, ci] = G w G^T (input-independent weight transform)
    u = np.einsum("xa,nb,oiab->xnoi", _G, _G, w)
    # arrange to [jo, nu, ci_p, xi, j, co_c]
    u6 = u.reshape(4, 4, NCO, 128, NCI, 128).transpose(2, 1, 5, 0, 4, 3)
    u_arr = np.ascontiguousarray(u6).astype(ml_dtypes.bfloat16)
    w2 = (w**2).sum(axis=(2, 3))  # [co, ci]
    # [ci_p, j, jo, co_c]
    w2_arr = np.ascontiguousarray(
        w2.reshape(NCO, 128, NCI, 128).transpose(3, 2, 0, 1)
    ).astype(ml_dtypes.bfloat16)
    in_maps = []
    # x parity split: [j, ci_p, parity, r, q]; parity0 = even cols,
    # parity1 = odd cols
    xb = x.reshape(B, NCI, 128, H, T, 2)
    for b in range(B):
        yb = np.empty((128, 2 * NCI), np.float32)
        yb[:, :NCI] = y_s[b].reshape(NCI, 128).T
        yb[:, NCI:] = bias.reshape(NCO, 128).T
        xs = np.ascontiguousarray(xb[b].transpose(0, 1, 4, 2, 3)).astype(
            ml_dtypes.bfloat16
        )
        in_maps.append({"x": xs, "yb": yb, "u": u_arr, "w2": w2_arr})
    return in_maps


def _install_trace_support():
    """Dev-only: register the axon NTFF profiling hook + disable the
    remote artifact upload so trace=True works in this container."""
    import sys
    import types

    import concourse.bass_utils as bu

    bu.upload_artifacts = lambda tmpdir: "local://" + str(tmpdir)
    if "antenv.axon_hooks" in sys.modules:
        return
    try:
        from trn_agent_boot.trn_boot import _ntff_profile_via_ctypes

        hook = _ntff_profile_via_ctypes("/opt/axon/libaxon_pjrt.so")
    except Exception:
        return
    mod = types.ModuleType("antenv.axon_hooks")
    mod.get_axon_ntff_profile_hook = lambda: hook
    mod.set_axon_ntff_profile_hook = lambda h: None
    sys.modules["antenv.axon_hooks"] = mod


def run(x, y_s, weight, bias, trace=False, tmpdir=None):
    nc = _get_nc()
    if trace:
        _install_trace_support()
    in_maps = _prep_inputs(x, y_s, weight, bias)
    res = run_bass_kernel_spmd(
        nc, in_maps, core_ids=list(range(B)), trace=trace, tmpdir=tmpdir
    )
    # descatter winograd layout: [jo, co_c, eta, mu, ti, tj] -> [co, h, w]
    out = np.stack(
        [
            res.results[b]["out"]
            .reshape(NCO, 128, 2, 2, T, T)
            .transpose(0, 1, 4, 2, 5, 3)
            .reshape(CO, H, W)
            for b in range(B)
        ]
    ).astype(np.float32)
    return out, res


def kernel(x, y_s, weight, bias):
    out, _ = run(
        np.asarray(x, dtype=np.float32),
        np.asarray(y_s, dtype=np.float32),
        np.asarray(weight, dtype=np.float32),
        np.asarray(bias, dtype=np.float32),
    )
    return out


# revision 9
# speedup vs baseline: 1.1645x; 1.0385x over previous
"""StyleGAN2-style modulated 3x3 conv (B=8, Ci=Co=512, H=W=32) on 8 TRN2 NeuronCores.

Sharding: data-parallel over batch, one sample per core (embarrassingly
parallel, no collectives).

Algorithm: Winograd F(2x2, 3x3). Per core:
  x ships pre-split by column parity (host): xt[j][par][r, q]
  mod:  parity planes P[par][j] = zero-padded(x * y_s)        (DVE TS)
  ct:   column-combine on parity planes (B^T d cols)          (DVE TT)
  v:    row-combine at stride-2 rows (B^T d B)                (DVE TT, 2x)
  M[xi,nu]   = sum_j U[xi,nu,j]^T @ V[j,xi,nu] (PE, fp32 PSUM, N=256)
  Y1[eta,nu] = xi-combine(M)                   (A^T M)        (DVE)
  Y[eta,mu]  = nu-combine(Y1)                  (A^T M A)      (DVE)
  ot[eta,mu,tile] = Y/rs + bias  (winograd layout; host descatters)
  rs = sqrt(sum_i ys2[i] w2[i,o] + eps)        demod (tiny PE matmuls)

Layout choices keep every hot DVE op on dense innermost runs so
tensor_tensor hits its 2x bf16 mode (the column-combine is inherently
one-element-offset and runs 1x, but on the smallest intermediate).
Output stays in winograd tile order so there is no strided scatter; the
host reshape is free.

Transformed weights U = G w G^T are computed host-side and streamed as
8 nu-paired 1MB slabs over three DMA queues in consumption order; demod
uses a separately shipped w2 = sum_k w_k^2.

Math note: the equal_lr scale s=(Ci*9)**-0.5 is folded out of both conv
and demod norm (eps compensated), so U/w2 come from the raw weights.
"""

import numpy as np
import ml_dtypes

import concourse.mybir as mybir
from concourse import bacc
from concourse.tile import TileContext
from concourse.bass_utils import run_bass_kernel_spmd

B = 8
CI = 512
CO = 512
H = W = 32
KK = 9
NCI = CI // 128
NCO = CO // 128
T = 16  # winograd tile grid (16x16 tiles of 2x2 outputs)
NPT = 256  # tiles per image = T*T
PR = 34  # padded rows
PW = 18  # parity-plane width (17 used + 1 pad for evenness)
EPS_EFF = 1e-8 * CI * KK

F32 = mybir.dt.float32
BF16 = mybir.dt.bfloat16
AF = mybir.ActivationFunctionType

# column-combine on parity planes. Plane 0 holds odd padded cols
# (p=2q+1, i.e. x even cols at idx 0..15, right-pad zero at idx 16);
# plane 1 holds even padded cols (p=2q: left-pad zero at idx 0, x odd
# cols at idx 1..16). Winograd col d_k = padded col 2tj+k:
#   d0 = plane1[tj], d1 = plane0[tj], d2 = plane1[tj+1], d3 = plane0[tj+1]
# nu0 = d0-d2, nu1 = d1+d2, nu2 = d2-d1, nu3 = d1-d3
# entries: (par_a, off_a, par_b, off_b, op)
CT_COMBINE = [
    (1, 0, 1, 1, "subtract"),
    (0, 0, 1, 1, "add"),
    (1, 1, 0, 0, "subtract"),
    (0, 0, 0, 1, "subtract"),
]
# row-combine (xi): rows a+2ti, b+2ti of ct
BT_ROWS = [(0, 2, "subtract"), (1, 2, "add"), (2, 1, "subtract"), (1, 3, "subtract")]


def build_nc():
    nc = bacc.Bacc("TRN2", target_bir_lowering=False, debug=False)

    # x: per j chunk [128, parity, 32, 16]
    x_ext = nc.declare_dram_parameter("x", [NCI, 128, 2, H, T], BF16, isOutput=False)
    yb_ext = nc.declare_dram_parameter("yb", [128, 2 * NCI], F32, isOutput=False)
    # transformed weights: [jo, nu, ci_p, xi, j, co_c]
    u_ext = nc.declare_dram_parameter(
        "u", [NCO, 4, 128, 4, NCI, 128], BF16, isOutput=False
    )
    # w2 = sum_k w_k^2: [ci_p, j, jo, co_c]
    w2_ext = nc.declare_dram_parameter("w2", [128, NCI, NCO, 128], BF16, isOutput=False)
    # out in winograd layout [jo, co_c, eta, mu, ti*16+tj], bf16 (host widens)
    out_ext = nc.declare_dram_parameter(
        "out", [NCO, 128, 2, 2, NPT], BF16, isOutput=True
    )

    with TileContext(nc) as tc:
        with (
            tc.tile_pool(name="singles", bufs=1) as singles,
            tc.tile_pool(name="us", bufs=12) as us,
            tc.tile_pool(name="big", bufs=1) as big,
            tc.tile_pool(name="xin", bufs=1) as xin,
            tc.tile_pool(name="y1s", bufs=1) as y1s,
            tc.tile_pool(name="tmps", bufs=2) as tmps,
            tc.tile_pool(name="outs", bufs=2) as outs,
            tc.tile_pool(name="cps", bufs=3, space="PSUM") as cps,
            tc.tile_pool(name="dps", bufs=1, space="PSUM") as dps,
            tc.tile_pool(name="wps", bufs=1, space="PSUM") as wps,
        ):
            xt_sb = [
                xin.tile([128, 2, H, T], BF16, tag=f"x{j}", name=f"xt{j}")
                for j in range(NCI)
            ]
            yb_sb = singles.tile([128, 2 * NCI], F32)
            w2_sb = singles.tile([128, NCI, NCO, 128], BF16)

            # parity planes + transform intermediates
            # pl[par][j]: [34 rows, 18]; ct[j, nu]: [34 rows, 16];
            # v[j, xi, nu]: [256]
            pl = big.tile([128, 2, NCI, PR, PW], BF16)
            ct_sb = big.tile([128, NCI, 4, PR, T], BF16)
            v_sb = big.tile([128, NCI, 4, 4, NPT], BF16)

            # ---- padding memsets first so the gpsimd queue clears them
            # before anything else (mod depends on them) ----
            nc.gpsimd.memset(pl[:, :, :, 0, :], 0.0)
            nc.gpsimd.memset(pl[:, :, :, PR - 1, :], 0.0)
            nc.gpsimd.memset(pl[:, 0, :, 1 : PR - 1, T], 0.0)
            nc.gpsimd.memset(pl[:, 1, :, 1 : PR - 1, 0], 0.0)

            # ---- input DMAs, priority order per queue ----
            u_sb = {}  # (jo, nu) -> [128, 4xi, 4j, 128co]

            def udma(engine, jo, nu):
                t = us.tile([128, 4, NCI, 128], BF16, tag="u", name=f"u{jo}{nu}")
                engine.dma_start(out=t, in_=u_ext[jo, nu])
                u_sb[(jo, nu)] = t

            # Two DMA queues only (a third just splits the same SDMA
            # bandwidth): even-jo slabs on sync, odd-jo on scalar, strictly
            # in consumption order k = 4*nu + jo. x gates the transform so
            # it goes first; w2 is only needed by demod (after round 0).
            nc.scalar.dma_start(out=yb_sb, in_=yb_ext[:, :])
            nc.sync.dma_start(out=xt_sb[0], in_=x_ext[0])
            nc.scalar.dma_start(out=xt_sb[2], in_=x_ext[2])
            nc.sync.dma_start(out=xt_sb[1], in_=x_ext[1])
            nc.scalar.dma_start(out=xt_sb[3], in_=x_ext[3])
            for nu in range(4):
                udma(nc.sync, 0, nu)
                udma(nc.scalar, 1, nu)
                if nu == 0:
                    nc.scalar.dma_start(out=w2_sb, in_=w2_ext[:, :])
                udma(nc.sync, 2, nu)
                udma(nc.scalar, 3, nu)

            # ---- PE warm-up (HAM clock gate) ----
            warm_lhs = singles.tile([128, 1], BF16)
            nc.vector.memset(warm_lhs, 1.0)
            warm_rhs = singles.tile([128, 512], BF16)
            nc.vector.memset(warm_rhs, 0.5)
            warm_ps = wps.tile([1, 512], F32)
            N_WARM = 16
            for i in range(N_WARM):
                nc.tensor.matmul(
                    out=warm_ps,
                    lhsT=warm_lhs,
                    rhs=warm_rhs,
                    start=(i == 0),
                    stop=(i == N_WARM - 1),
                )

            eps_sb = singles.tile([128, 1], F32)
            nc.vector.memset(eps_sb, EPS_EFF)

            def mod(j):
                # plane0 (data at idx 0..15) from x even cols; plane1 at 1..16
                nc.vector.tensor_scalar(
                    out=pl[:, 0, j, 1 : H + 1, 0:T],
                    in0=xt_sb[j][:, 0],
                    scalar1=yb_sb[:, j : j + 1],
                    scalar2=None,
                    op0=mybir.AluOpType.mult,
                )
                nc.vector.tensor_scalar(
                    out=pl[:, 1, j, 1 : H + 1, 1 : T + 1],
                    in0=xt_sb[j][:, 1],
                    scalar1=yb_sb[:, j : j + 1],
                    scalar2=None,
                    op0=mybir.AluOpType.mult,
                )

            def ctop(nu, j=None):
                pa, oa, pb, ob, op = CT_COMBINE[nu]
                jj = slice(None) if j is None else slice(j, j + 1)
                nc.vector.tensor_tensor(
                    out=ct_sb[:, jj, nu],
                    in0=pl[:, pa, jj, :, oa : oa + T],
                    in1=pl[:, pb, jj, :, ob : ob + T],
                    op=getattr(mybir.AluOpType, op),
                )

            def vop(xi, nu):
                a, b, op = BT_ROWS[xi]
                nc.vector.tensor_tensor(
                    out=v_sb[:, :, xi, nu],
                    in0=ct_sb[:, :, nu, a : min(a + 2 * T, PR) : 2, :],
                    in1=ct_sb[:, :, nu, b : min(b + 2 * T, PR) : 2, :],
                    op=getattr(mybir.AluOpType, op),
                )

            for j in range(NCI):
                mod(j)
                ctop(0, j)
            ys2_sb = singles.tile([128, NCI], BF16)
            nc.vector.tensor_mul(ys2_sb, yb_sb[:, 0:NCI], yb_sb[:, 0:NCI])
            for xi in range(4):
                vop(xi, 0)
            for nu in range(1, 4):
                ctop(nu)
                for xi in range(4):
                    vop(xi, nu)

            rs_sb = singles.tile([128, NCO], F32)

            # per-jo evacuated M (bf16): [128, 4nu, 4xi, 256]
            m_sb = [
                y1s.tile([128, 4, 4, NPT], BF16, tag=f"m_{jo}", name=f"m_{jo}")
                for jo in range(NCO)
            ]
            # per-jo Y1 (bf16): [128, 2eta, 4nu, 256]
            y1_sb = [
                y1s.tile([128, 2, 4, NPT], BF16, tag=f"y1_{jo}", name=f"y1_{jo}")
                for jo in range(NCO)
            ]
            # per-jo Y (bf16): [128, 2eta, 2mu, 256]
            yt_sb = [
                y1s.tile([128, 2, 2, NPT], BF16, tag=f"yt_{jo}", name=f"yt_{jo}")
                for jo in range(NCO)
            ]
            ot_sb = [
                outs.tile([128, 2, 2, NPT], BF16, tag=f"ot{jo % 2}", name=f"ot{jo}")
                for jo in range(NCO)
            ]

            def unit(jo, nu):
                # M[xi] for this (jo, nu): 16 matmuls N=256, xi-outer so each
                # xi's accumulation chain is contiguous; one ACT op evacuates
                # all 4 xi slices to SBUF bf16.
                ps = cps.tile([128, 4, NPT], F32, tag="ups")
                u = u_sb[(jo, nu)]
                for xi in range(4):
                    for j in range(NCI):
                        nc.tensor.matmul(
                            out=ps[:, xi],
                            lhsT=u[:, xi, j],
                            rhs=v_sb[:, j, xi, nu],
                            start=(j == 0),
                            stop=(j == NCI - 1),
                        )
                nc.scalar.activation(out=m_sb[jo][:, nu], in_=ps, func=AF.Copy)

            TT = nc.vector.tensor_tensor
            ADD = mybir.AluOpType.add
            SUB = mybir.AluOpType.subtract

            def stage1(jo, n0, n1):
                # Y1[0,nu] = M0+M1+M2 ; Y1[1,nu] = M1-M2-M3 over nu in [n0,n1)
                m = m_sb[jo]
                y1 = y1_sb[jo]
                t = tmps.tile([128, 4, NPT], BF16, tag="t1", name="t")
                TT(out=t[:, n0:n1], in0=m[:, n0:n1, 0], in1=m[:, n0:n1, 1], op=ADD)
                TT(out=y1[:, 0, n0:n1], in0=t[:, n0:n1], in1=m[:, n0:n1, 2], op=ADD)
                TT(out=t[:, n0:n1], in0=m[:, n0:n1, 1], in1=m[:, n0:n1, 2], op=SUB)
                TT(out=y1[:, 1, n0:n1], in0=t[:, n0:n1], in1=m[:, n0:n1, 3], op=SUB)

            def stage2(jo, mu):
                # Y[:,mu0] = Y1n0+Y1n1+Y1n2 ; Y[:,mu1] = Y1n1-Y1n2-Y1n3
                y1 = y1_sb[jo]
                yt = yt_sb[jo]
                op = ADD if mu == 0 else SUB
                na, nb, ncol = (0, 1, 2) if mu == 0 else (1, 2, 3)
                t = tmps.tile([128, 2, NPT], BF16, tag="t2", name="t")
                TT(out=t, in0=y1[:, :, na], in1=y1[:, :, nb], op=op)
                TT(out=yt[:, :, mu], in0=t, in1=y1[:, :, ncol], op=op)

            def finish(jo, mu):
                # ot[:, :, mu] = yt[:, :, mu]*rs + bias (winograd layout, no
                # strided scatter); ACT takes eta=0 and DVE eta=1
                yt = yt_sb[jo]
                ot = ot_sb[jo]
                nc.scalar.activation(
                    out=ot[:, 0, mu],
                    in_=yt[:, 0, mu],
                    func=AF.Identity,
                    bias=yb_sb[:, NCI + jo : NCI + jo + 1],
                    scale=rs_sb[:, jo : jo + 1],
                )
                nc.vector.tensor_scalar(
                    out=ot[:, 1, mu],
                    in0=yt[:, 1, mu],
                    scalar1=rs_sb[:, jo : jo + 1],
                    scalar2=yb_sb[:, NCI + jo : NCI + jo + 1],
                    op0=mybir.AluOpType.mult,
                    op1=mybir.AluOpType.add,
                )

            # ---- PE stream: nu-outer rounds; per-jo combine work staggered:
            # nu0-2 parts after round 2, nu3 parts + output after each
            # round-3 unit ----
            for jo in range(NCO):
                unit(jo, 0)
            # demod: PE matmuls + sqrt + reciprocal (w2 arrives mid round 0)
            xs2_ps = dps.tile([128, NCO], F32)
            for jo in range(NCO):
                for j in range(NCI):
                    nc.tensor.matmul(
                        out=xs2_ps[:, jo : jo + 1],
                        lhsT=w2_sb[:, j, jo],
                        rhs=ys2_sb[:, j : j + 1],
                        start=(j == 0),
                        stop=(j == NCI - 1),
                    )
            nc.scalar.activation(out=rs_sb, in_=xs2_ps, func=AF.Sqrt, bias=eps_sb)
            nc.vector.reciprocal(out=rs_sb, in_=rs_sb)
            for jo in range(NCO):
                unit(jo, 1)
            for jo in range(NCO):
                unit(jo, 2)
                stage1(jo, 0, 3)
                stage2(jo, 0)
                finish(jo, 0)
            out_whole_q = [nc.scalar, nc.sync, nc.scalar]
            for jo in range(NCO):
                unit(jo, 3)
                stage1(jo, 3, 4)
                stage2(jo, 1)
                finish(jo, 1)
                if jo < 3:
                    out_whole_q[jo].dma_start(out=out_ext[jo], in_=ot_sb[jo])
                else:
                    nc.scalar.dma_start(out=out_ext[jo][:, 0], in_=ot_sb[jo][:, 0])
                    nc.sync.dma_start(out=out_ext[jo][:, 1], in_=ot_sb[jo][:, 1])

            warm_sink = singles.tile([1, 1], F32)
            nc.vector.tensor_copy(out=warm_sink, in_=warm_ps[0:1, 0:1])
    nc.compile()
    return nc


_NC_CACHE = None


def _get_nc():
    global _NC_CACHE
    if _NC_CACHE is None:
        _NC_CACHE = build_nc()
    return _NC_CACHE


_G = np.array(
    [[1, 0, 0], [0.5, 0.5, 0.5], [0.5, -0.5, 0.5], [0, 0, 1]], np.float64
)


def _prep_inputs(x, y_s, weight, bias):
    w = weight.astype(np.float64)
    # U[xi, nu, co, ci] = G w G^T (input-independent weight transform)
    u = np.einsum("xa,nb,oiab->xnoi", _G, _G, w)
    # arrange to [jo, nu, ci_p, xi, j, co_c]
    u6 = u.reshape(4, 4, NCO, 128, NCI, 128).transpose(2, 1, 5, 0, 4, 3)
    u_arr = np.ascontiguousarray(u6).astype(ml_dtypes.bfloat16)
    w2 = (w**2).sum(axis=(2, 3))  # [co, ci]
    # [ci_p, j, jo, co_c]
    w2_arr = np.ascontiguousarray(
        w2.reshape(NCO, 128, NCI, 128).transpose(3, 2, 0, 1)
    ).astype(ml_dtypes.bfloat16)
    in_maps = []
    # x parity split: [j, ci_p, parity, r, q]; parity0 = even cols,
    # parity1 = odd cols
    xb = x.reshape(B, NCI, 128, H, T, 2)
    for b in range(B):
        yb = np.empty((128, 2 * NCI), np.float32)
        yb[:, :NCI] = y_s[b].reshape(NCI, 128).T
        yb[:, NCI:] = bias.reshape(NCO, 128).T
        xs = np.ascontiguousarray(xb[b].transpose(0, 1, 4, 2, 3)).astype(
            ml_dtypes.bfloat16
        )
        in_maps.append({"x": xs, "yb": yb, "u": u_arr, "w2": w2_arr})
    return in_maps


def _install_trace_support():
    """Dev-only: register the axon NTFF profiling hook + disable the
    remote artifact upload so trace=True works in this container."""
    import sys
    import types

    import concourse.bass_utils as bu

    bu.upload_artifacts = lambda tmpdir: "local://" + str(tmpdir)
    if "antenv.axon_hooks" in sys.modules:
        return
    try:
        from trn_agent_boot.trn_boot import _ntff_profile_via_ctypes

        hook = _ntff_profile_via_ctypes("/opt/axon/libaxon_pjrt.so")
    except Exception:
        return
    mod = types.ModuleType("antenv.axon_hooks")
    mod.get_axon_ntff_profile_hook = lambda: hook
    mod.set_axon_ntff_profile_hook = lambda h: None
    sys.modules["antenv.axon_hooks"] = mod


def run(x, y_s, weight, bias, trace=False, tmpdir=None):
    nc = _get_nc()
    if trace:
        _install_trace_support()
    in_maps = _prep_inputs(x, y_s, weight, bias)
    res = run_bass_kernel_spmd(
        nc, in_maps, core_ids=list(range(B)), trace=trace, tmpdir=tmpdir
    )
    # descatter winograd layout: [jo, co_c, eta, mu, ti, tj] -> [co, h, w]
    out = np.stack(
        [
            res.results[b]["out"]
            .reshape(NCO, 128, 2, 2, T, T)
            .transpose(0, 1, 4, 2, 5, 3)
            .reshape(CO, H, W)
            for b in range(B)
        ]
    ).astype(np.float32)
    return out, res


def kernel(x, y_s, weight, bias):
    out, _ = run(
        np.asarray(x, dtype=np.float32),
        np.asarray(y_s, dtype=np.float32),
        np.asarray(weight, dtype=np.float32),
        np.asarray(bias, dtype=np.float32),
    )
    return out
